# revision 30
# baseline (speedup 1.0000x reference)
"""CapsNet dynamic-routing kernel for 8 TRN2 NeuronCores.

Problem: x [256,1152,8], W [1152,10,8,16], 3 routing iterations, out [256,10,16,1].

Strategy v2 (replicated iteration 1 + I-sharded iterations 2-3):
  Collective cost model (measured): a CC barrier runs ~12-35us at every
  execution start; the FIRST collective op then pays ~11us of firmware
  staging from its doorbell plus a first-op duration premium; later ops
  start ~1.2us after their doorbell (AR 82KB ~11us, RS ~9.5us).  The old
  3-collective I-sharded baseline serialized [AR + 20us segment] x2 + RS.

  Here iteration 1 (uniform c = 1/O) is computed REPLICATED on every core
  from the full x and W:
    s1[b,(o,e)] = x_flat[b,:] @ W_flat[:,(o,e)]      (K = 9216, all I)
  This needs no communication and rides behind a 3-queue chunked DMA
  stream of xT_full (4.7MB) + W_full (2.9MB) bf16 (~290GB/s aggregate,
  done ~46us) that overlaps the CC barrier window.  Iterations 2-3 run
  I-sharded (s-partial matmul with K = local 1152, one AllReduce for
  iter 2's s, one ReduceScatter for iter 3's s): TWO collectives total
  and ONE post-collective serial segment instead of three.
  A tiny (2KB) dummy AllReduce issued at kernel start absorbs the
  first-collective staging+premium entirely off the critical path (it
  finishes ~56us, before the real AR's doorbell at ~66us; in the old
  baseline whose first doorbell rang at ~28us a dummy only serialized).
  The s1 PE accumulation runs as contiguous start/stop groups per batch
  tile (33+33+6 k-tiles, alternating bt at group boundaries): the PE
  trails the DMA stream, and interleaving two accumulation groups
  instruction-by-instruction within one PSUM tile corrupts the sums.

  Host-side ROTATED k-tile layout: each core receives xT_full / W_full with
  its own 9 local k-tiles rolled to positions 0..8, so the I-sharded
  iterations address tiles [0,9) on every core (pure SPMD, no dynamic
  indexing, no duplicate loads -- the local slices are views into the full
  tensors).

  Per routing iteration (I-sharded part, local (i,d) rows = 1152):
    s_partial[b,(o,e)] = x_flat[b,:] @ (c*W)_flat[:,(o,e)]
    s = AllReduce(s_partial) over the 8 I-shards               (82KB bf16)
    v = squash(s)            (computed redundantly on all cores)
    G[(i,d),(o,e)] = x_flat^T @ v_flat
    agree[i,o] = (1/B) * sum_{d,e} (W_flat * G)[(i,d),(o,e)]   (local)
    b += agree ; c = softmax(b, axis=o)                        (local)
  The final iteration needs only a batch shard of v per core, so it uses
  ReduceScatter and the host concatenates the 8 output shards.
  Matmul operands are bf16 (fp32 matmul runs 2 passes at 1/4 rate on
  TRN2); accumulation stays fp32 (PSUM); collectives carry bf16.
  Softmax skips max-subtraction: |b| stays O(1) for this routing.
  ACT table prefetches read a constant tile (Mblk) -- reading a live
  tensor (ex) created a WAR hazard that serialized the softmax behind a
  1.3us table load in the previous version.
"""

import os

import numpy as np

_DBG = bool(os.environ.get("DBG_BQ"))
_SIMPLE_DMA = bool(os.environ.get("SIMPLE_DMA"))

B, I, O, DIN, DOUT = 256, 1152, 10, 8, 16
NCORES = 8
I_SH = I // NCORES          # 144 input capsules per core
ID = I_SH * DIN             # 1152 local (i,d) rows
NT = ID // 128              # 9 partition tiles of local (i,d)
KT = (I * DIN) // 128       # 72 partition tiles of full (i,d)
BT = B // 128               # 2 partition tiles of batch
OE = O * DOUT               # 160
ROUTING_ITERS = 3
CHK = 12                    # k-tiles per DMA chunk of the full tensors
NCH = KT // CHK             # 6 chunks

_CACHE = {}


def _bc(ap_mod, ap, n):
    """View an AP with an extra innermost broadcast axis of length n."""
    return ap_mod.AP(tensor=ap.tensor, offset=ap.offset, ap=[*ap.ap, [0, n]])


def _build():
    import concourse.bass as bass
    import concourse.bacc as bacc
    import concourse.tile as tile
    from concourse import mybir

    f32 = mybir.dt.float32
    bf16 = mybir.dt.bfloat16
    AF = mybir.ActivationFunctionType
    ALU = mybir.AluOpType

    nc = bacc.Bacc("TRN2", target_bir_lowering=False, debug=False,
                   num_devices=NCORES)

    # All inputs pre-tiled on host to [128, ...] so every DMA is contiguous.
    # xTf/Wbf are the FULL tensors, k-tiles rotated so the core's local
    # shard sits at tiles [0, NT).
    xTf_d = nc.dram_tensor("xTf", [128, KT, B], bf16, kind="ExternalInput")
    Wbf_d = nc.dram_tensor("Wbf", [128, KT, OE], bf16, kind="ExternalInput")
    xf_d = nc.dram_tensor("xf", [128, BT, ID], bf16, kind="ExternalInput")
    MB_d = nc.dram_tensor("Mblk", [128, 128], bf16, kind="ExternalInput")
    # final iteration uses ReduceScatter: each core emits 16 partitions
    # x [BT, OE] (batch rows bt*128 + 16*rank + p)
    PSH = 128 // NCORES
    out_d = nc.dram_tensor("out", [PSH, BT, OE], f32, kind="ExternalOutput")

    with tile.TileContext(nc) as tc:
        with (
            tc.tile_pool(name="sb", bufs=1) as sb,
            tc.tile_pool(name="work", bufs=2) as work,
            tc.tile_pool(name="ps_s", bufs=2, space="PSUM") as ps_s,
            tc.tile_pool(name="ps_g", bufs=2, space="PSUM") as ps_g,
            tc.tile_pool(name="ps_a", bufs=2, space="PSUM") as ps_a,
            tc.tile_pool(name="dram", bufs=3, space="DRAM") as dram,
        ):
            # ---- persistent SBUF tensors ----
            xTf = sb.tile([128, KT, B], bf16)     # x_flat^T tiles (lhsT for s)
            Wbf = sb.tile([128, KT, OE], bf16)    # W_flat bf16 (full)
            xf = sb.tile([128, BT, ID], bf16)     # local x_flat (lhsT for G)
            Mblk = sb.tile([128, 128], bf16)      # 8x8 block-diag ones
            bq = sb.tile([128, NT, O], f32)       # routing logits b (expanded)
            Wc = sb.tile([128, NT, OE], bf16)     # c * W (local)
            s_sb = sb.tile([128, BT, OE], bf16)   # local partial s (bounce)
            sf = sb.tile([128, BT, OE], bf16)     # all-reduced s
            vb = sb.tile([128, BT, OE], bf16)     # squash(s) bf16 (rhs for G)
            WG = sb.tile([128, NT, OE], bf16)     # W * G
            A1 = sb.tile([128, NT, O], bf16)      # e-reduced agreement
            dmy = sb.tile([128, 1], f32)          # ACT table-prefetch scratch

            # ---- input DMA: spread the 8.3MB across the three DMA queues
            # (sync + scalar HWDGE, gpsimd SWDGE), chunk-interleaved so the
            # full-K s1 matmul can consume k-tiles as they land.  Balanced
            # ~2.8MB/queue; xf (needed only by G1, ~4us after s1) rides the
            # queue tails split in 3 so G groups can start in order.
            qs = [nc.sync, nc.scalar, nc.gpsimd]

            def chunk(dst, src, c):
                return (dst[:, c * CHK:(c + 1) * CHK, :],
                        src[:, c * CHK:(c + 1) * CHK, :])

            nc.gpsimd.dma_start(out=Mblk[:], in_=MB_d[:])
            if _SIMPLE_DMA:
                nc.sync.dma_start(out=xTf[:], in_=xTf_d[:])
                nc.sync.dma_start(out=Wbf[:], in_=Wbf_d[:])
                nc.gpsimd.dma_start(out=xf[:], in_=xf_d[:])
            else:
                # (A_c = xTf chunk c ~786KB, B_c = Wbf chunk c ~492KB)
                plan = [
                    (nc.sync,   [("A", 0), ("B", 2), ("A", 3), ("B", 5)]),
                    (nc.scalar, [("B", 0), ("A", 1), ("B", 3), ("A", 4)]),
                    (nc.gpsimd, [("B", 1), ("A", 2), ("B", 4), ("A", 5)]),
                ]
                for q, items in plan:
                    for kind, c in items:
                        if kind == "A":
                            d, s = chunk(xTf, xTf_d, c)
                        else:
                            d, s = chunk(Wbf, Wbf_d, c)
                        q.dma_start(out=d, in_=s)
                # xf in 3 column-chunks (G group g needs tiles [3g, 3g+3))
                XCH = ID // 3
                for g in range(3):
                    qs[g].dma_start(
                        out=xf[:, :, g * XCH:(g + 1) * XCH],
                        in_=xf_d[:, :, g * XCH:(g + 1) * XCH])

            # Pre-load the Sqrt ACT table during setup (table loads are
            # ~1.3us each and otherwise land on the critical chain).
            nc.scalar.activation(out=dmy[:], in_=Mblk[:, 0:1], func=AF.Sqrt)

            # Early dummy AllReduce (2KB): the first collective of an
            # execution pays ~11us of firmware staging after the CC
            # barrier.  Here the real first collective's doorbell rings
            # only at ~60us (after the replicated iteration 1), so a tiny
            # collective issued now absorbs the staging (and the
            # first-op premium) entirely off the critical path.  (In the
            # old I-sharded-iter1 design the real doorbell rang at ~28us,
            # before the dummy could finish — there it only serialized.)
            wcc_in = nc.inline_tensor(
                np.zeros((128, 4), dtype=np.float32), name="wcc_in")
            wcc_out = dram.tile([128, 4], f32, tag="wcc_out",
                                addr_space="Shared")
            nc.gpsimd.collective_compute(
                "AllReduce", ALU.add,
                replica_groups=[list(range(NCORES))],
                ins=[wcc_in.ap().opt()], outs=[wcc_out.opt()])

            Wb4 = Wbf.rearrange("p t (o e) -> p t o e", o=O)
            Wc4 = Wc.rearrange("p t (o e) -> p t o e", o=O)
            sf4 = sf.rearrange("p b (o e) -> p b o e", o=O)
            vb4 = vb.rearrange("p b (o e) -> p b o e", o=O)

            for it in range(ROUTING_ITERS):
                first, last = it == 0, it == ROUTING_ITERS - 1

                if first:
                    # Replicated full-K s1 (uniform c folded into the squash
                    # scale): k-outer so the PE trails the DMA stream by one
                    # chunk; both batch tiles accumulate in one PSUM tile.
                    # PE accumulation groups must be CONTIGUOUS in the
                    # instruction stream (interleaving two start/stop groups
                    # corrupts the sums).  Split each bt's K=72 sum into 3
                    # contiguous groups, alternating bt at group boundaries
                    # so the PE still trails the DMA chunk stream; the last
                    # group is small to minimize the post-DMA PE tail.
                    GRPS = (33, 33, 6)
                    s1_ps = [ps_g.tile([128, 3, OE], f32, tag="g_ps",
                                       name=f"s1_ps{bt}")
                             for bt in range(BT)]
                    k0 = 0
                    for gi, gn in enumerate(GRPS):
                        for bt in range(BT):
                            for kk in range(gn):
                                k = k0 + kk
                                nc.tensor.matmul(
                                    s1_ps[bt][:, gi, :],
                                    xTf[:, k, bt * 128:(bt + 1) * 128],
                                    Wbf[:, k, :],
                                    start=(kk == 0), stop=(kk == gn - 1))
                        k0 += gn
                    # combine the 3 partial groups + cast to bf16 in one
                    # strided reduce per bt (gpsimd can't read PSUM, so
                    # both stay on Vector)
                    with nc.allow_low_precision("s1 bf16 like the AR path"):
                        for bt in range(BT):
                            gp = s1_ps[bt]
                            gview = bass.AP(
                                tensor=gp.tensor, offset=gp.offset,
                                ap=[gp.ap[0], [1, OE], [OE, 3]])
                            nc.vector.reduce_sum(out=sf[:, bt, :], in_=gview,
                                                 axis=mybir.AxisListType.X)
                    if _DBG:
                        dbg2_d = nc.dram_tensor("dbg2", [128, BT, OE], bf16,
                                                kind="ExternalOutput")
                        nc.sync.dma_start(out=dbg2_d[:], in_=sf[:])

                    # squash; c=1 carried as s_raw = O*s_true:
                    #   v = s_raw*(1/O^2)*sqrt(ss_raw)/(1+ss_raw/O^2)
                    sq = work.tile([128, BT, OE], f32, tag="sq")
                    nc.vector.tensor_tensor(out=sq[:], in0=sf[:],
                                            in1=sf[:], op=ALU.mult)
                    ss = work.tile([128, BT, O], f32, tag="ss")
                    nc.vector.reduce_sum(
                        out=ss[:],
                        in_=sq.rearrange("p b (o e) -> p b o e", o=O),
                        axis=mybir.AxisListType.X)
                    t1 = work.tile([128, BT, O], f32, tag="t1")
                    nc.scalar.activation(out=t1[:], in_=ss[:], func=AF.Sqrt)
                    # Exp-table prefetch for softmax2, PINNED after this
                    # squash's Sqrt by reading t1 and feeding the next Exp's
                    # scale: exp(0*t1) = 1, used as scale=1 -- exact, and
                    # the scheduler can no longer hoist it early (unpinned
                    # dmy ops all scheduled at t~21-30, thrashing tables).
                    dmyE = work.tile([128, 1], f32, tag="dmyE")
                    nc.scalar.activation(out=dmyE[:], in_=t1[:, 0, 0:1],
                                         func=AF.Exp, scale=0.0)
                    den = work.tile([128, BT, O], f32, tag="den")
                    nc.vector.tensor_scalar(
                        out=den[:], in0=ss[:], scalar1=1.0 / (O * O),
                        scalar2=1.0, op0=ALU.mult, op1=ALU.add)
                    nc.vector.reciprocal(out=den[:], in_=den[:])
                    rat = work.tile([128, BT, O], f32, tag="rat")
                    nc.vector.scalar_tensor_tensor(
                        out=rat[:], in0=t1[:], scalar=1.0 / (O * O),
                        in1=den[:], op0=ALU.mult, op1=ALU.mult)
                    nc.vector.tensor_tensor(
                        out=vb4[:], in0=sf4[:],
                        in1=_bc(bass, rat[:], DOUT), op=ALU.mult)
                else:
                    # c = softmax(b) over o per (i,d) row; |b| is O(1) so no
                    # max-subtraction is needed.  scale = exp(0) = 1 from
                    # the previous iteration's pinned Exp prefetch.
                    ex = work.tile([128, NT, O], f32, tag="ex")
                    nc.scalar.activation(out=ex[:], in_=bq[:], func=AF.Exp,
                                         scale=dmyE[:, 0:1])
                    # Sqrt-table prefetch for this iteration's squash,
                    # pinned after the Exp by reading ex; sqrt(0*ex) = 0 is
                    # later the squash Sqrt's bias (exact).  It loads the
                    # table during the s-matmul + collective slack.
                    dmyS = work.tile([128, 1], f32, tag="dmyS")
                    nc.scalar.activation(out=dmyS[:], in_=ex[:, 0, 0:1],
                                         func=AF.Sqrt, scale=0.0)
                    sm = work.tile([128, NT], f32, tag="sm")
                    nc.vector.reduce_sum(out=sm[:], in_=ex[:],
                                         axis=mybir.AxisListType.X)
                    nc.vector.reciprocal(out=sm[:], in_=sm[:])
                    nc.vector.tensor_tensor(
                        out=ex[:], in0=ex[:], in1=_bc(bass, sm[:], O),
                        op=ALU.mult)
                    # Wc = c * W in ramped chunks so the first s-matmuls
                    # start as early as possible; alternate Vector/GpSimd
                    # so two chunks compute concurrently
                    g0 = 0
                    for ci, gw in enumerate((1, 2, 3, 3)):
                        eng = nc.gpsimd if ci % 2 else nc.vector
                        eng.tensor_tensor(
                            out=Wc4[:, g0:g0 + gw],
                            in0=_bc(bass, ex[:, g0:g0 + gw, :], DOUT),
                            in1=Wb4[:, g0:g0 + gw], op=ALU.mult)
                        g0 += gw

                    # s_partial = x_flat @ Wc : out [b-tile 128, OE].  The
                    # bounce buffers are partition-major [128, BT, OE] so
                    # every DMA hop moves contiguous chunks.  Both bt groups
                    # accumulate in ONE PSUM tile (1280B < 2KB bank) with
                    # contiguous start/stop groups, so a single PSUM->SBUF
                    # copy (on the otherwise-idle Scalar engine) and a
                    # single bounce DMA feed the collective.
                    cc_in = dram.tile([128, BT, OE], bf16, tag="cc_in")
                    s_ps = ps_s.tile([128, BT, OE], f32, tag="s_ps")
                    for bt in range(BT):
                        for k in range(NT):
                            nc.tensor.matmul(
                                s_ps[:, bt, :],
                                xTf[:, k, bt * 128:(bt + 1) * 128],
                                Wc[:, k, :],
                                start=(k == 0), stop=(k == NT - 1))
                    nc.scalar.activation(out=s_sb[:], in_=s_ps[:],
                                         func=AF.Copy)
                    # bounce on gpsimd so the collective's doorbell
                    # (also gpsimd) follows in-queue without an extra
                    # cross-engine semaphore hop
                    nc.gpsimd.dma_start(out=cc_in[:], in_=s_sb[:])

                    if last:
                        # Final iteration: each core only needs 1/8 of v, so
                        # ReduceScatter; the shard is 16 partitions x
                        # [BT, OE] (batch rows bt*128 + 16*rank + p); the
                        # host reassembles.
                        cc_rs = dram.tile([PSH, BT, OE], bf16, tag="cc_rs")
                        nc.gpsimd.collective_compute(
                            "ReduceScatter", ALU.add,
                            replica_groups=[list(range(NCORES))],
                            ins=[cc_in.opt()], outs=[cc_rs.opt()])
                        # land the RS result via the scalar queue and run
                        # the squash's square in-queue right behind it, so
                        # the cross-engine DMA wakeup overlaps real work
                        s3 = sb.tile([PSH, BT, OE], bf16)
                        nc.scalar.dma_start(out=s3[:], in_=cc_rs[:])
                        sq3 = work.tile([PSH, BT, OE], f32, tag="sq3")
                        nc.scalar.activation(out=sq3[:], in_=s3[:],
                                             func=AF.Square)
                        ss3 = work.tile([PSH, BT, O], f32, tag="ss3")
                        nc.vector.reduce_sum(
                            out=ss3[:],
                            in_=sq3.rearrange("p b (o e) -> p b o e", o=O),
                            axis=mybir.AxisListType.X)
                        # bias = this iteration's pinned sqrt-prefetch (0)
                        t13 = work.tile([PSH, BT, O], f32, tag="t13")
                        nc.scalar.activation(out=t13[:], in_=ss3[:],
                                             func=AF.Sqrt,
                                             bias=dmyS[:PSH, 0:1])
                        den3 = work.tile([PSH, BT, O], f32, tag="den3")
                        nc.vector.tensor_scalar_add(den3[:], ss3[:], 1.0)
                        nc.vector.reciprocal(out=den3[:], in_=den3[:])
                        rat3 = work.tile([PSH, BT, O], f32, tag="rat3")
                        nc.vector.tensor_tensor(out=rat3[:], in0=t13[:],
                                                in1=den3[:], op=ALU.mult)
                        v3 = work.tile([PSH, BT, OE], f32, tag="v3")
                        nc.vector.tensor_tensor(
                            out=v3.rearrange("p b (o e) -> p b o e", o=O),
                            in0=s3.rearrange("p b (o e) -> p b o e", o=O),
                            in1=_bc(bass, rat3[:], DOUT), op=ALU.mult)
                        nc.sync.dma_start(out=out_d[:], in_=v3[:])
                        continue

                    # AllReduce s over the 8 I-shards
                    cc_out = dram.tile([128, BT, OE], bf16, tag="cc_out",
                                       addr_space="Shared")
                    nc.gpsimd.collective_compute(
                        "AllReduce", ALU.add,
                        replica_groups=[list(range(NCORES))],
                        ins=[cc_in.opt()], outs=[cc_out.opt()])
                    # land via the scalar queue; the first squash op (the
                    # square) runs in-queue on Scalar right behind it, so
                    # the ~2us cross-engine DMA-completion wakeup overlaps
                    # useful work instead of gating the whole chain
                    nc.scalar.dma_start(out=sf[:], in_=cc_out[:])

                    # squash: v = s * sqrt(ss)/(1+ss) per (b, o)
                    sq = work.tile([128, BT, OE], f32, tag="sq")
                    nc.scalar.activation(out=sq[:], in_=sf[:],
                                         func=AF.Square)
                    ss = work.tile([128, BT, O], f32, tag="ss")
                    nc.vector.reduce_sum(
                        out=ss[:],
                        in_=sq.rearrange("p b (o e) -> p b o e", o=O),
                        axis=mybir.AxisListType.X)
                    # bias = sqrt-prefetch output (0): table already warm
                    t1 = work.tile([128, BT, O], f32, tag="t1")
                    nc.scalar.activation(out=t1[:], in_=ss[:], func=AF.Sqrt,
                                         bias=dmyS[:, 0:1])
                    den = work.tile([128, BT, O], f32, tag="den")
                    nc.vector.tensor_scalar_add(den[:], ss[:], 1.0)
                    nc.vector.reciprocal(out=den[:], in_=den[:])
                    rat = work.tile([128, BT, O], f32, tag="rat")
                    nc.vector.tensor_tensor(out=rat[:], in0=t1[:],
                                            in1=den[:], op=ALU.mult)
                    # Exp-table prefetch for the next softmax, pinned after
                    # this squash's Sqrt (exp(0*t1) = 1 feeds its scale);
                    # rides the agreement-path slack
                    dmyE = work.tile([128, 1], f32, tag="dmyE")
                    nc.scalar.activation(out=dmyE[:], in_=t1[:, 0, 0:1],
                                         func=AF.Exp, scale=0.0)
                    nc.vector.tensor_tensor(
                        out=vb4[:], in0=sf4[:],
                        in1=_bc(bass, rat[:], DOUT), op=ALU.mult)

                # G = x_flat^T @ v ; agree = (1/B) sum_de W*G ; b += agree.
                # Three (i,d)-tiles share one PSUM bank (3*640B < 2KB) so
                # the W*G multiply and e-reduction run once per group.
                # (gpsimd cannot read PSUM, so this chain stays on Vector.)
                GW = 3
                for gi, g in enumerate(range(0, NT, GW)):
                    g_ps = ps_g.tile([128, GW, OE], f32, tag="g_ps")
                    for j in range(GW):
                        for bt in range(BT):
                            nc.tensor.matmul(
                                g_ps[:, j, :],
                                xf[:, bt, (g + j) * 128:(g + j + 1) * 128],
                                vb[:, bt, :],
                                start=(bt == 0), stop=(bt == BT - 1))
                    nc.vector.tensor_tensor(
                        out=WG[:, g:g + GW, :], in0=Wbf[:, g:g + GW, :],
                        in1=g_ps[:], op=ALU.mult)
                    with nc.allow_low_precision("agreement tolerates bf16"):
                        nc.vector.reduce_sum(
                            out=A1[:, g:g + GW, :],
                            in_=WG[:, g:g + GW, :].rearrange(
                                "p g (o e) -> p (g o) e", o=O),
                            axis=mybir.AxisListType.X)
                # d-sums of all NT tiles in ONE matmul (rhs free = NT*O=90)
                a_ps = ps_a.tile([128, NT, O], f32, tag="a_ps")
                nc.tensor.matmul(a_ps[:, :, :], Mblk[:],
                                 A1[:, :, :], start=True, stop=True)
                if first:
                    nc.vector.tensor_scalar_mul(bq[:], a_ps[:], 1.0 / B)
                    if _DBG:
                        dbg_d = nc.dram_tensor("dbg", [128, NT, O], f32,
                                               kind="ExternalOutput")
                        nc.sync.dma_start(out=dbg_d[:], in_=bq[:])
                else:
                    nc.vector.scalar_tensor_tensor(
                        out=bq[:], in0=a_ps[:], scalar=1.0 / B,
                        in1=bq[:], op0=ALU.mult, op1=ALU.add)

    nc.compile()
    return nc


def _get_nc():
    if "nc" not in _CACHE:
        _CACHE["nc"] = _build()
    return _CACHE["nc"]


def _tile128(a):
    """[R, C] -> [128, R//128, C] with row r = t*128+p at [p, t]."""
    r, c = a.shape
    return np.ascontiguousarray(
        a.reshape(r // 128, 128, c).transpose(1, 0, 2))


def _make_in_maps(x, W):
    from concourse import mybir
    bfdt = mybir.dt.np(mybir.dt.bfloat16)
    x = np.asarray(x, dtype=np.float32)
    W = np.asarray(W, dtype=np.float32)
    mblk = np.kron(np.eye(16, dtype=np.float32),
                   np.ones((8, 8), dtype=np.float32)).astype(bfdt)
    x_flat = x.reshape(B, I * DIN)
    w_flat = W.transpose(0, 2, 1, 3).reshape(I * DIN, OE)
    xT_t = _tile128(np.ascontiguousarray(x_flat.T)).astype(bfdt)  # [128,72,B]
    wb_t = _tile128(w_flat).astype(bfdt)                          # [128,72,OE]
    in_maps = []
    for core in range(NCORES):
        isl = slice(core * I_SH, (core + 1) * I_SH)
        xf_loc = x[:, isl, :].reshape(B, ID)
        in_maps.append({
            # k-tiles rolled so the core's local shard is tiles [0, NT)
            "xTf": np.ascontiguousarray(np.roll(xT_t, -NT * core, axis=1)),
            "Wbf": np.ascontiguousarray(np.roll(wb_t, -NT * core, axis=1)),
            "xf": _tile128(xf_loc).astype(bfdt),
            "Mblk": mblk,
        })
    return in_maps


def _ensure_ntff_hook():
    """This image's antenv lacks axon_hooks; reconstruct it so trace=True
    can reach the NTFF profiler in libaxon_pjrt.so."""
    import sys
    import types
    try:
        import antenv.axon_hooks  # noqa: F401
        return
    except ImportError:
        pass
    try:
        import antenv
        from trn_agent_boot.trn_boot import _ntff_profile_via_ctypes
        hook = _ntff_profile_via_ctypes("/opt/axon/libaxon_pjrt.so")
        mod = types.ModuleType("antenv.axon_hooks")
        mod._hook = hook
        mod.get_axon_ntff_profile_hook = lambda: mod._hook
        mod.set_axon_ntff_profile_hook = (
            lambda h: setattr(mod, "_hook", h))
        sys.modules["antenv.axon_hooks"] = mod
        antenv.axon_hooks = mod
    except Exception as e:  # profiling is best-effort
        print("ntff hook setup failed:", e)


def _run_hw(x, W, trace=False, **kwargs):
    from concourse import bass_utils
    if trace:
        _ensure_ntff_hook()
    nc = _get_nc()
    res = bass_utils.run_bass_kernel_spmd(
        nc, _make_in_maps(x, W), core_ids=list(range(NCORES)),
        trace=trace, **kwargs)
    shards = np.stack([res.results[c]["out"] for c in range(NCORES)])
    return _assemble(shards), res


def _assemble(shards):
    """shards [NCORES, 16, BT, OE] -> full [B, O, DOUT, 1]; core r's shard
    holds batch rows bt*128 + 16*r + p (ReduceScatter partition sharding)."""
    shards = np.asarray(shards, dtype=np.float32).reshape(
        NCORES, 128 // NCORES, BT, OE)
    # [r, p, bt, f] -> [bt, r, p, f] -> [B, OE]
    full = shards.transpose(2, 0, 1, 3).reshape(B, OE)
    return full.reshape(B, O, DOUT)[..., None]


def kernel(x, W):
    out, _ = _run_hw(x, W, trace=False)
    return out

